# revision 18
# baseline (speedup 1.0000x reference)
"""AWPINN wavelet-PINN kernel for 8x Trainium2 NeuronCores (Bass/Tile).

Math: for each point i and wavelet k (N=65536, K=512):
  xt = wx*x - bx (same y,z);  s = xt^2+yt^2+zt^2;  E = exp(-0.5*s)
  W  = xt*yt*zt*E          (reference's xw*yw*zw = -W)
  output = sum_k (-coeff*scale)_k * W + bias
  d2u_dx2 = sum_k (coeff*scale*wx^2)_k * (3 - xt^2) * W   (same y,z)

Device structure:
  - s and T3=xt*yt*zt are low-rank bilinear forms in per-point features
    F = [x2,y2,z2,xyz,xy,xz,yz,x,y,z,1] -> TensorEngine matmuls
    (contraction = features, M = 128 wavelets/block, FD = 512 points).
  - All matmuls fp16 (1 cycle/column on PE; fp32/fp32r are ~3x slower).
    Near-fp32 precision via hi/lo splits stacked along the contraction dim:
    [Lh;Ll;Lh] @ [Fh;Fh;Fl] == L @ F with ~2^-21 products. The constant
    feature's lo-row is dropped -> exactly 32 rows, so the four feature
    matmuls of a k-block pair rotate over the PE's four 32-row groups
    (tile_position) and their weight loads overlap the previous matmul.
  - d2 terms decompose via xt^2 = wx^2*x^2 - 2*wx*bx*x + bx^2 into 3
    matvec columns each -> one [128k, 10] output matmul per k-block
    (fp16 hi only; the rel-err budget is ~2e-2, fp16 weights give ~1e-3).
  - Point-chunks are processed in PAIRS sharing one stationary set:
    matmuls are ordered stationary-major so each Ls/Lt quadrant tile is
    loaded once per pair of chunks, halving LDWEIGHTS traffic.
  - The d2 recombination (x^2*R1 + x*R2 + R3 etc.) runs on the HOST:
    the device ships the 10 reduced rows R[10, NP] straight to DRAM,
    eliminating the on-device epilogue + DRAM transpose bounce entirely.
Data parallel over points: each core handles 8192 points; no collectives.
"""

import numpy as np

N_TOTAL = 65536
K_TOTAL = 512
N_CORES = 8
NP_CORE = N_TOTAL // N_CORES        # 8192 points per core
CHUNK = 512                         # points per matmul (PSUM bank = 512 fp32)
N_CHUNKS = NP_CORE // CHUNK         # 16
N_GROUPS = N_CHUNKS // 2            # chunk pairs sharing stationary loads
KBLK = K_TOTAL // 128               # 4 wavelet blocks of 128
NFEAT = 11                          # features per point
NST = 32                            # stacked contraction rows (ones-lo dropped)

_COMPILED = {}


def _split16(a):
    """Split fp32 into fp16 hi + fp16 lo (hi+lo carries ~21 mantissa bits)."""
    a = np.ascontiguousarray(a, np.float32)
    hi = a.astype(np.float16)
    lo = np.float32(a - hi.astype(np.float32)).astype(np.float16)
    return hi, lo


def _stack32(L):
    """[11,n] fp32 coeffs -> [32,n] fp16 stack [Lh; Ll; Lh[:10]]."""
    Lh, Ll = _split16(L)
    return np.concatenate([Lh, Ll, Lh[:NFEAT - 1]], axis=0)


def _build_program():
    import concourse.bacc as bacc
    import concourse.mybir as mybir
    import concourse.tile as tile

    f32 = mybir.dt.float32
    f16 = mybir.dt.float16
    AF = mybir.ActivationFunctionType

    nc = bacc.Bacc("TRN2", target_bir_lowering=False, debug=False)

    # fst: feature stack, host-replicated to all four 32-row groups
    fst_d = nc.dram_tensor("fst", [4 * NST, NP_CORE], f16, kind="ExternalInput")
    # lst: rows 0-31 Ls-stack, 32-63 Lt-stack, 64-95 Ls, 96-127 Lt;
    # columns grouped by k-block
    lst_d = nc.dram_tensor("lst", [128, K_TOTAL], f16, kind="ExternalInput")
    loh_d = nc.dram_tensor("loh", [128, KBLK * 10], f16, kind="ExternalInput")
    out_d = nc.dram_tensor("out", [10, NP_CORE], f32, kind="ExternalOutput")

    with tile.TileContext(nc) as tc:
        with (
            tc.tile_pool(name="persist", bufs=1) as pp,
            tc.tile_pool(name="epool", bufs=4) as ep,
            tc.tile_pool(name="psum_s", bufs=4, space="PSUM") as psps,
            tc.tile_pool(name="psum_t", bufs=3, space="PSUM") as pspt,
            tc.tile_pool(name="psum_out", bufs=1, space="PSUM") as pso,
            tc.tile_pool(name="wpool", bufs=24) as wpool,
        ):
            lst_t = pp.tile([128, K_TOTAL], f16, tag="lst")
            nc.gpsimd.dma_start(lst_t[:], lst_d[:])
            loh_t = pp.tile([128, KBLK * 10], f16, tag="loh")
            nc.scalar.dma_start(loh_t[:], loh_d[:])

            # persistent feature stack (host-replicated across row groups);
            # point-slices fan out over three DMA queues so early chunks are
            # never starved
            f_all = pp.tile([4 * NST, NP_CORE], f16, tag="f_all")
            bounds = [0, CHUNK, 2 * CHUNK, 4 * CHUNK] + [
                q * NP_CORE // 8 for q in range(3, 9)]
            engs = [nc.sync, nc.gpsimd, nc.scalar, nc.sync, nc.gpsimd,
                    nc.sync, nc.gpsimd, nc.sync, nc.gpsimd]
            for q in range(len(bounds) - 1):
                qs = slice(bounds[q], bounds[q + 1])
                engs[q].dma_start(f_all[:, qs], fst_d[:, qs])

            # warm the EXP activation-table load during the initial DMAs so
            # it is off the critical path of the first real exp
            warm = pp.tile([128, 1], f32, tag="warm")
            nc.gpsimd.memset(warm[:], 0.0)
            nc.scalar.activation(warm[:], warm[:], AF.Exp, scale=-0.5)

            # reduced rows R, staged in SBUF then DMA'd out per chunk-pair
            r_rows = pp.tile([10, NP_CORE], f32, tag="r_rows")

            pending = []   # (group, w tiles, po tile or None)

            def emit_outs(kbs):
                # emit one half of the previous group's output matmuls;
                # chunk cA accumulates at PE col group 0 (psum partitions
                # 0-9), cB at col group 32 (partitions 32-41): the paired
                # output matmuls run concurrently in the array and both
                # fit one PSUM bank
                if not pending:
                    return
                t0, w_ts, po = pending[-1]
                if po is None:
                    po = pso.tile([42, CHUNK], f32, tag="po", name=f"po{t0}")
                    pending[-1] = (t0, w_ts, po)
                po_j = (po[0:10, :], po[32:42, :])
                for kb in kbs:
                    for j in range(2):
                        nc.tensor.matmul(
                            po_j[j],
                            loh_t[:, kb * 10:(kb + 1) * 10],
                            w_ts[kb][j],
                            start=(kb == 0), stop=(kb == KBLK - 1),
                            tile_position=(0, 32 * j))
                if kbs[-1] != KBLK - 1:
                    return
                pending.pop()
                # drain: one chunk each on scalar and vector (gpsimd
                # cannot read PSUM)
                nc.scalar.copy(
                    r_rows[:, 2 * t0 * CHUNK:(2 * t0 + 1) * CHUNK], po_j[0])
                nc.vector.tensor_copy(
                    r_rows[:, (2 * t0 + 1) * CHUNK:(2 * t0 + 2) * CHUNK],
                    po_j[1])
                nc.sync.dma_start(
                    out_d[:, 2 * t0 * CHUNK:(2 * t0 + 2) * CHUNK],
                    r_rows[:, 2 * t0 * CHUNK:(2 * t0 + 2) * CHUNK])

            for t in range(N_GROUPS):
                cA, cB = 2 * t, 2 * t + 1
                fA = f_all[:, cA * CHUNK:(cA + 1) * CHUNK]
                fB = f_all[:, cB * CHUNK:(cB + 1) * CHUNK]
                w_ts = [[None, None] for _ in range(KBLK)]
                for p in range(KBLK // 2):
                    kbs = (2 * p, 2 * p + 1)
                    # interleave the two k-blocks of a pair so consecutive
                    # matmuls rotate through all four PE row groups
                    # (q0,q1,q2,q3) -> up to 4 matmuls in flight at once
                    xs = {}
                    ys = {}
                    for kb in kbs:
                        for j in range(2):
                            xs[kb, j] = psps.tile(
                                [128, CHUNK], f32, tag="ps_s",
                                name=f"s{t}_{kb}_{j}")
                            ys[kb, j] = pspt.tile(
                                [128, CHUNK], f32, tag="ps_t",
                                name=f"t{t}_{kb}_{j}")
                    for j, f_t in ((0, fA), (1, fB)):
                        for kb in kbs:
                            gs, gt = 2 * (kb % 2), 2 * (kb % 2) + 1
                            ks = slice(kb * 128, (kb + 1) * 128)
                            nc.tensor.matmul(
                                xs[kb, j][:],
                                lst_t[32 * gs:32 * (gs + 1), ks],
                                f_t[32 * gs:32 * (gs + 1), :],
                                start=True, stop=True,
                                tile_position=(32 * gs, 0))
                            nc.tensor.matmul(
                                ys[kb, j][:],
                                lst_t[32 * gt:32 * (gt + 1), ks],
                                f_t[32 * gt:32 * (gt + 1), :],
                                start=True, stop=True,
                                tile_position=(32 * gt, 0))
                    for j in range(2):
                        for kb in kbs:
                            e_t = ep.tile([128, CHUNK], f32, tag="e",
                                          name=f"e{t}_{kb}_{j}")
                            nc.scalar.activation(
                                e_t[:], xs[kb, j][:], AF.Exp, scale=-0.5)
                            w_t = wpool.tile([128, CHUNK], f16, tag="w",
                                             name=f"w{t}_{kb}_{j}")
                            nc.vector.tensor_mul(
                                w_t[:], ys[kb, j][:], e_t[:])
                            w_ts[kb][j] = w_t[:]
                    # previous group's output matmuls fill the PE while this
                    # block's exp/W-mul chain catches up
                    emit_outs([2 * p, 2 * p + 1])
                pending.append((t, w_ts, None))
            emit_outs([0, 1])
            emit_outs([2, 3])
    nc.compile()
    return nc


def _get_program():
    if "nc" not in _COMPILED:
        _COMPILED["nc"] = _build_program()
    return _COMPILED["nc"]


def _host_prep(x, y, z, wx, bx, wy, by, wz, bz, coeff):
    """Build per-core input maps (features + coefficient matrices)."""
    f8 = np.float64
    wx64, bx64 = wx.astype(f8), bx.astype(f8)
    wy64, by64 = wy.astype(f8), by.astype(f8)
    wz64, bz64 = wz.astype(f8), bz.astype(f8)
    c64 = coeff.astype(f8)
    sc = np.sqrt(np.clip(wx64 * wy64 * wz64, 1e-12, None))
    Z = np.zeros_like(wx64)

    # s = xt^2 + yt^2 + zt^2 over features [x2,y2,z2,xyz,xy,xz,yz,x,y,z,1]
    Ls = np.stack([
        wx64 ** 2, wy64 ** 2, wz64 ** 2, Z, Z, Z, Z,
        -2 * wx64 * bx64, -2 * wy64 * by64, -2 * wz64 * bz64,
        bx64 ** 2 + by64 ** 2 + bz64 ** 2,
    ]).astype(np.float32)                      # [11, K]
    # T3 = xt*yt*zt
    Lt = np.stack([
        Z, Z, Z,
        wx64 * wy64 * wz64, -wx64 * wy64 * bz64, -wx64 * by64 * wz64,
        -bx64 * wy64 * wz64, wx64 * by64 * bz64, bx64 * wy64 * bz64,
        bx64 * by64 * wz64, -bx64 * by64 * bz64,
    ]).astype(np.float32)                      # [11, K]
    b1 = c64 * sc * wx64 ** 2
    b2 = c64 * sc * wy64 ** 2
    b3 = c64 * sc * wz64 ** 2
    Lo = np.stack([
        -c64 * sc,
        -b1 * wx64 ** 2, 2 * b1 * wx64 * bx64, b1 * (3 - bx64 ** 2),
        -b2 * wy64 ** 2, 2 * b2 * wy64 * by64, b2 * (3 - by64 ** 2),
        -b3 * wz64 ** 2, 2 * b3 * wz64 * bz64, b3 * (3 - bz64 ** 2),
    ], axis=1).astype(np.float32)              # [K, 10]

    Ls32 = _stack32(Ls)                        # [32, K] fp16
    Lt32 = _stack32(Lt)
    lst_pack = np.concatenate([Ls32, Lt32, Ls32, Lt32], axis=0)  # [128, K]
    Loh = Lo.astype(np.float16)
    loh_pack = np.concatenate(
        [Loh[kb * 128:(kb + 1) * 128] for kb in range(KBLK)], axis=1)  # [128, 40]

    in_maps = []
    for cid in range(N_CORES):
        sl = slice(cid * NP_CORE, (cid + 1) * NP_CORE)
        xs, ys, zs = (np.ascontiguousarray(a[sl], np.float32) for a in (x, y, z))
        F = np.stack([
            xs * xs, ys * ys, zs * zs, xs * ys * zs, xs * ys, xs * zs,
            ys * zs, xs, ys, zs, np.ones_like(xs),
        ]).astype(np.float32)                  # [11, NP_CORE]
        Fh, Fl = _split16(F)
        fst1 = np.concatenate([Fh, Fh, Fl[:NFEAT - 1]], axis=0)  # [32, NP]
        fst = np.concatenate([fst1] * 4, axis=0)                 # [128, NP]
        in_maps.append({"fst": fst, "lst": lst_pack, "loh": loh_pack})
    return in_maps


def _run_device(in_maps, trace=False):
    from concourse.bass_utils import run_bass_kernel_spmd
    nc = _get_program()
    last_err = None
    for _attempt in range(3):
        try:
            return run_bass_kernel_spmd(
                nc, in_maps, list(range(N_CORES)), trace=trace)
        except Exception as ex:  # transient NRT device errors recover on retry
            last_err = ex
    raise last_err


def kernel(x, y, z, wx, bx, wy, by, wz, bz, coeff, bias, _trace=False):
    x, y, z = (np.asarray(a, np.float32) for a in (x, y, z))
    in_maps = _host_prep(
        x, y, z,
        *(np.asarray(a, np.float32) for a in (wx, bx, wy, by, wz, bz, coeff)))
    res = _run_device(in_maps, trace=_trace)
    R = np.concatenate(
        [res.results[cid]["out"] for cid in range(N_CORES)], axis=1)  # [10, N]
    bias_f = np.float32(np.asarray(bias))
    x64, y64, z64 = (a.astype(np.float64) for a in (x, y, z))
    R64 = R.astype(np.float64)
    output = (R64[0] + np.float64(bias_f)).astype(np.float32)
    d2x = (x64 * x64 * R64[1] + x64 * R64[2] + R64[3]).astype(np.float32)
    d2y = (y64 * y64 * R64[4] + y64 * R64[5] + R64[6]).astype(np.float32)
    d2z = (z64 * z64 * R64[7] + z64 * R64[8] + R64[9]).astype(np.float32)
    if _trace:
        kernel._last_results = res
    return (output, d2x, d2y, d2z)


# revision 19
# speedup vs baseline: 1.1499x; 1.1499x over previous
"""AWPINN wavelet-PINN kernel for 8x Trainium2 NeuronCores (Bass/Tile).

Math: for each point i and wavelet k (N=65536, K=512):
  xt = wx*x - bx (same y,z);  s = xt^2+yt^2+zt^2;  E = exp(-0.5*s)
  W  = xt*yt*zt*E          (reference's xw*yw*zw = -W)
  output = sum_k (-coeff*scale)_k * W + bias
  d2u_dx2 = sum_k (coeff*scale*wx^2)_k * (3 - xt^2) * W   (same y,z)

Device structure:
  - s and T3=xt*yt*zt are low-rank bilinear forms in per-point features
    F = [x2,y2,z2,xyz,xy,xz,yz,x,y,z,1] -> TensorEngine matmuls
    (contraction = features, M = 128 wavelets/block, FD = 512 points).
  - All matmuls fp16 (1 cycle/column on PE; fp32/fp32r are ~3x slower).
    Near-fp32 precision via hi/lo splits stacked along the contraction dim:
    [Lh;Ll;Lh] @ [Fh;Fh;Fl] == L @ F with ~2^-21 products. The constant
    feature's lo-row is dropped -> exactly 32 rows, so the four feature
    matmuls of a k-block pair rotate over the PE's four 32-row groups
    (tile_position) and their weight loads overlap the previous matmul.
  - d2 terms decompose via xt^2 = wx^2*x^2 - 2*wx*bx*x + bx^2 into 3
    matvec columns each -> one [128k, 10] output matmul per k-block
    (fp16 hi only; the rel-err budget is ~2e-2, fp16 weights give ~1e-3).
  - Point-chunks are processed in PAIRS sharing one stationary set:
    matmuls are ordered stationary-major so each Ls/Lt quadrant tile is
    loaded once per pair of chunks, halving LDWEIGHTS traffic.
  - The d2 recombination (x^2*R1 + x*R2 + R3 etc.) runs on the HOST:
    the device ships the 10 reduced rows R[10, NP] straight to DRAM,
    eliminating the on-device epilogue + DRAM transpose bounce entirely.
Data parallel over points: each core handles 8192 points; no collectives.
"""

import numpy as np

N_TOTAL = 65536
K_TOTAL = 512
N_CORES = 8
NP_CORE = N_TOTAL // N_CORES        # 8192 points per core
CHUNK = 512                         # points per matmul (PSUM bank = 512 fp32)
N_CHUNKS = NP_CORE // CHUNK         # 16
N_GROUPS = N_CHUNKS // 2            # chunk pairs sharing stationary loads
KBLK = K_TOTAL // 128               # 4 wavelet blocks of 128
NFEAT = 11                          # features per point
NST = 32                            # stacked contraction rows (ones-lo dropped)

_COMPILED = {}


def _split16(a):
    """Split fp32 into fp16 hi + fp16 lo (hi+lo carries ~21 mantissa bits)."""
    a = np.ascontiguousarray(a, np.float32)
    hi = a.astype(np.float16)
    lo = np.float32(a - hi.astype(np.float32)).astype(np.float16)
    return hi, lo


def _stack32(L):
    """[11,n] fp32 coeffs -> [32,n] fp16 stack [Lh; Ll; Lh[:10]]."""
    Lh, Ll = _split16(L)
    return np.concatenate([Lh, Ll, Lh[:NFEAT - 1]], axis=0)


def _build_program():
    import concourse.bacc as bacc
    import concourse.mybir as mybir
    import concourse.tile as tile

    f32 = mybir.dt.float32
    f16 = mybir.dt.float16
    AF = mybir.ActivationFunctionType

    nc = bacc.Bacc("TRN2", target_bir_lowering=False, debug=False)

    # fst: feature stack, host-replicated to all four 32-row groups
    fst_d = nc.dram_tensor("fst", [4 * NST, NP_CORE], f16, kind="ExternalInput")
    # lst: rows 0-31 Ls-stack, 32-63 Lt-stack, 64-95 Ls, 96-127 Lt;
    # columns grouped by k-block
    lst_d = nc.dram_tensor("lst", [128, K_TOTAL], f16, kind="ExternalInput")
    loh_d = nc.dram_tensor("loh", [128, KBLK * 10], f16, kind="ExternalInput")
    out_d = nc.dram_tensor("out", [10, NP_CORE], f32, kind="ExternalOutput")

    with tile.TileContext(nc) as tc:
        with (
            tc.tile_pool(name="persist", bufs=1) as pp,
            tc.tile_pool(name="epool", bufs=4) as ep,
            tc.tile_pool(name="psum_s", bufs=4, space="PSUM") as psps,
            tc.tile_pool(name="psum_t", bufs=3, space="PSUM") as pspt,
            tc.tile_pool(name="psum_out", bufs=1, space="PSUM") as pso,
            tc.tile_pool(name="wpool", bufs=24) as wpool,
        ):
            lst_t = pp.tile([128, K_TOTAL], f16, tag="lst")
            nc.gpsimd.dma_start(lst_t[:, 0:K_TOTAL // 2],
                                lst_d[:, 0:K_TOTAL // 2])
            nc.gpsimd.dma_start(lst_t[:, K_TOTAL // 2:],
                                lst_d[:, K_TOTAL // 2:])
            loh_t = pp.tile([128, KBLK * 10], f16, tag="loh")
            nc.scalar.dma_start(loh_t[:], loh_d[:])

            # persistent feature stack (host-replicated across row groups);
            # point-slices fan out over three DMA queues so early chunks are
            # never starved
            f_all = pp.tile([4 * NST, NP_CORE], f16, tag="f_all")
            bounds = [0, CHUNK, 2 * CHUNK, 4 * CHUNK] + [
                q * NP_CORE // 8 for q in range(3, 9)]
            engs = [nc.sync, nc.gpsimd, nc.scalar, nc.sync, nc.gpsimd,
                    nc.sync, nc.gpsimd, nc.sync, nc.gpsimd]
            for q in range(len(bounds) - 1):
                qs = slice(bounds[q], bounds[q + 1])
                engs[q].dma_start(f_all[:, qs], fst_d[:, qs])

            # warm the EXP activation-table load during the initial DMAs so
            # it is off the critical path of the first real exp
            warm = pp.tile([128, 1], f32, tag="warm")
            nc.gpsimd.memset(warm[:], 0.0)
            nc.scalar.activation(warm[:], warm[:], AF.Exp, scale=-0.5)

            # reduced rows R, staged in SBUF then DMA'd out per chunk-pair
            r_rows = pp.tile([10, NP_CORE], f32, tag="r_rows")

            pending = []   # (group, w tiles, po tile or None)

            def emit_outs(kbs):
                # emit one half of the previous group's output matmuls;
                # chunk cA accumulates at PE col group 0 (psum partitions
                # 0-9), cB at col group 32 (partitions 32-41): the paired
                # output matmuls run concurrently in the array and both
                # fit one PSUM bank
                if not pending:
                    return
                t0, w_ts, po = pending[-1]
                if po is None:
                    po = pso.tile([42, CHUNK], f32, tag="po", name=f"po{t0}")
                    pending[-1] = (t0, w_ts, po)
                po_j = (po[0:10, :], po[32:42, :])
                for kb in kbs:
                    for j in range(2):
                        nc.tensor.matmul(
                            po_j[j],
                            loh_t[:, kb * 10:(kb + 1) * 10],
                            w_ts[kb][j],
                            start=(kb == 0), stop=(kb == KBLK - 1),
                            tile_position=(0, 32 * j))
                if kbs[-1] != KBLK - 1:
                    return
                pending.pop()
                # drain: one chunk each on scalar and vector (gpsimd
                # cannot read PSUM)
                nc.scalar.copy(
                    r_rows[:, 2 * t0 * CHUNK:(2 * t0 + 1) * CHUNK], po_j[0])
                nc.vector.tensor_copy(
                    r_rows[:, (2 * t0 + 1) * CHUNK:(2 * t0 + 2) * CHUNK],
                    po_j[1])
                nc.sync.dma_start(
                    out_d[:, 2 * t0 * CHUNK:(2 * t0 + 1) * CHUNK],
                    r_rows[:, 2 * t0 * CHUNK:(2 * t0 + 1) * CHUNK])
                nc.sync.dma_start(
                    out_d[:, (2 * t0 + 1) * CHUNK:(2 * t0 + 2) * CHUNK],
                    r_rows[:, (2 * t0 + 1) * CHUNK:(2 * t0 + 2) * CHUNK])

            for t in range(N_GROUPS):
                cA, cB = 2 * t, 2 * t + 1
                fA = f_all[:, cA * CHUNK:(cA + 1) * CHUNK]
                fB = f_all[:, cB * CHUNK:(cB + 1) * CHUNK]
                w_ts = [[None, None] for _ in range(KBLK)]
                for p in range(KBLK // 2):
                    kbs = (2 * p, 2 * p + 1)
                    # interleave the two k-blocks of a pair so consecutive
                    # matmuls rotate through all four PE row groups
                    # (q0,q1,q2,q3) -> up to 4 matmuls in flight at once
                    xs = {}
                    ys = {}
                    for kb in kbs:
                        for j in range(2):
                            xs[kb, j] = psps.tile(
                                [128, CHUNK], f32, tag="ps_s",
                                name=f"s{t}_{kb}_{j}")
                            ys[kb, j] = pspt.tile(
                                [128, CHUNK], f32, tag="ps_t",
                                name=f"t{t}_{kb}_{j}")
                    for j, f_t in ((0, fA), (1, fB)):
                        for kb in kbs:
                            gs, gt = 2 * (kb % 2), 2 * (kb % 2) + 1
                            ks = slice(kb * 128, (kb + 1) * 128)
                            nc.tensor.matmul(
                                xs[kb, j][:],
                                lst_t[32 * gs:32 * (gs + 1), ks],
                                f_t[32 * gs:32 * (gs + 1), :],
                                start=True, stop=True,
                                tile_position=(32 * gs, 0))
                            nc.tensor.matmul(
                                ys[kb, j][:],
                                lst_t[32 * gt:32 * (gt + 1), ks],
                                f_t[32 * gt:32 * (gt + 1), :],
                                start=True, stop=True,
                                tile_position=(32 * gt, 0))
                    for j in range(2):
                        for kb in kbs:
                            e_t = ep.tile([128, CHUNK], f32, tag="e",
                                          name=f"e{t}_{kb}_{j}")
                            nc.scalar.activation(
                                e_t[:], xs[kb, j][:], AF.Exp, scale=-0.5)
                            w_t = wpool.tile([128, CHUNK], f16, tag="w",
                                             name=f"w{t}_{kb}_{j}")
                            nc.vector.tensor_mul(
                                w_t[:], ys[kb, j][:], e_t[:])
                            w_ts[kb][j] = w_t[:]
                    # previous group's output matmuls fill the PE while this
                    # block's exp/W-mul chain catches up
                    emit_outs([2 * p, 2 * p + 1])
                pending.append((t, w_ts, None))
            emit_outs([0, 1])
            emit_outs([2, 3])
    nc.compile()
    return nc


def _get_program():
    if "nc" not in _COMPILED:
        _COMPILED["nc"] = _build_program()
    return _COMPILED["nc"]


def _host_prep(x, y, z, wx, bx, wy, by, wz, bz, coeff):
    """Build per-core input maps (features + coefficient matrices)."""
    f8 = np.float64
    wx64, bx64 = wx.astype(f8), bx.astype(f8)
    wy64, by64 = wy.astype(f8), by.astype(f8)
    wz64, bz64 = wz.astype(f8), bz.astype(f8)
    c64 = coeff.astype(f8)
    sc = np.sqrt(np.clip(wx64 * wy64 * wz64, 1e-12, None))
    Z = np.zeros_like(wx64)

    # s = xt^2 + yt^2 + zt^2 over features [x2,y2,z2,xyz,xy,xz,yz,x,y,z,1]
    Ls = np.stack([
        wx64 ** 2, wy64 ** 2, wz64 ** 2, Z, Z, Z, Z,
        -2 * wx64 * bx64, -2 * wy64 * by64, -2 * wz64 * bz64,
        bx64 ** 2 + by64 ** 2 + bz64 ** 2,
    ]).astype(np.float32)                      # [11, K]
    # T3 = xt*yt*zt
    Lt = np.stack([
        Z, Z, Z,
        wx64 * wy64 * wz64, -wx64 * wy64 * bz64, -wx64 * by64 * wz64,
        -bx64 * wy64 * wz64, wx64 * by64 * bz64, bx64 * wy64 * bz64,
        bx64 * by64 * wz64, -bx64 * by64 * bz64,
    ]).astype(np.float32)                      # [11, K]
    b1 = c64 * sc * wx64 ** 2
    b2 = c64 * sc * wy64 ** 2
    b3 = c64 * sc * wz64 ** 2
    Lo = np.stack([
        -c64 * sc,
        -b1 * wx64 ** 2, 2 * b1 * wx64 * bx64, b1 * (3 - bx64 ** 2),
        -b2 * wy64 ** 2, 2 * b2 * wy64 * by64, b2 * (3 - by64 ** 2),
        -b3 * wz64 ** 2, 2 * b3 * wz64 * bz64, b3 * (3 - bz64 ** 2),
    ], axis=1).astype(np.float32)              # [K, 10]

    Ls32 = _stack32(Ls)                        # [32, K] fp16
    Lt32 = _stack32(Lt)
    lst_pack = np.concatenate([Ls32, Lt32, Ls32, Lt32], axis=0)  # [128, K]
    Loh = Lo.astype(np.float16)
    loh_pack = np.concatenate(
        [Loh[kb * 128:(kb + 1) * 128] for kb in range(KBLK)], axis=1)  # [128, 40]

    in_maps = []
    for cid in range(N_CORES):
        sl = slice(cid * NP_CORE, (cid + 1) * NP_CORE)
        xs, ys, zs = (np.ascontiguousarray(a[sl], np.float32) for a in (x, y, z))
        F = np.stack([
            xs * xs, ys * ys, zs * zs, xs * ys * zs, xs * ys, xs * zs,
            ys * zs, xs, ys, zs, np.ones_like(xs),
        ]).astype(np.float32)                  # [11, NP_CORE]
        Fh, Fl = _split16(F)
        fst1 = np.concatenate([Fh, Fh, Fl[:NFEAT - 1]], axis=0)  # [32, NP]
        fst = np.concatenate([fst1] * 4, axis=0)                 # [128, NP]
        in_maps.append({"fst": fst, "lst": lst_pack, "loh": loh_pack})
    return in_maps


def _run_device(in_maps, trace=False):
    from concourse.bass_utils import run_bass_kernel_spmd
    nc = _get_program()
    last_err = None
    for _attempt in range(3):
        try:
            return run_bass_kernel_spmd(
                nc, in_maps, list(range(N_CORES)), trace=trace)
        except Exception as ex:  # transient NRT device errors recover on retry
            last_err = ex
    raise last_err


def kernel(x, y, z, wx, bx, wy, by, wz, bz, coeff, bias, _trace=False):
    x, y, z = (np.asarray(a, np.float32) for a in (x, y, z))
    in_maps = _host_prep(
        x, y, z,
        *(np.asarray(a, np.float32) for a in (wx, bx, wy, by, wz, bz, coeff)))
    res = _run_device(in_maps, trace=_trace)
    R = np.concatenate(
        [res.results[cid]["out"] for cid in range(N_CORES)], axis=1)  # [10, N]
    bias_f = np.float32(np.asarray(bias))
    x64, y64, z64 = (a.astype(np.float64) for a in (x, y, z))
    R64 = R.astype(np.float64)
    output = (R64[0] + np.float64(bias_f)).astype(np.float32)
    d2x = (x64 * x64 * R64[1] + x64 * R64[2] + R64[3]).astype(np.float32)
    d2y = (y64 * y64 * R64[4] + y64 * R64[5] + R64[6]).astype(np.float32)
    d2z = (z64 * z64 * R64[7] + z64 * R64[8] + R64[9]).astype(np.float32)
    if _trace:
        kernel._last_results = res
    return (output, d2x, d2y, d2z)


# revision 20
# speedup vs baseline: 1.1534x; 1.0031x over previous
"""AWPINN wavelet-PINN kernel for 8x Trainium2 NeuronCores (Bass/Tile).

Math: for each point i and wavelet k (N=65536, K=512):
  xt = wx*x - bx (same y,z);  s = xt^2+yt^2+zt^2;  E = exp(-0.5*s)
  W  = xt*yt*zt*E          (reference's xw*yw*zw = -W)
  output = sum_k (-coeff*scale)_k * W + bias
  d2u_dx2 = sum_k (coeff*scale*wx^2)_k * (3 - xt^2) * W   (same y,z)

Device structure:
  - s and T3=xt*yt*zt are low-rank bilinear forms in per-point features
    F = [x2,y2,z2,xyz,xy,xz,yz,x,y,z,1] -> TensorEngine matmuls
    (contraction = features, M = 128 wavelets/block, FD = 512 points).
  - All matmuls fp16 (1 cycle/column on PE; fp32/fp32r are ~3x slower).
    Near-fp32 precision via hi/lo splits stacked along the contraction dim:
    [Lh;Ll;Lh] @ [Fh;Fh;Fl] == L @ F with ~2^-21 products. The constant
    feature's lo-row is dropped -> exactly 32 rows, so the four feature
    matmuls of a k-block pair rotate over the PE's four 32-row groups
    (tile_position) and their weight loads overlap the previous matmul.
  - d2 terms decompose via xt^2 = wx^2*x^2 - 2*wx*bx*x + bx^2 into 3
    matvec columns each -> one [128k, 10] output matmul per k-block
    (fp16 hi only; the rel-err budget is ~2e-2, fp16 weights give ~1e-3).
  - Point-chunks are processed in PAIRS sharing one stationary set:
    matmuls are ordered stationary-major so each Ls/Lt quadrant tile is
    loaded once per pair of chunks, halving LDWEIGHTS traffic.
  - The d2 recombination (x^2*R1 + x*R2 + R3 etc.) runs on the HOST:
    the device ships the 10 reduced rows R[10, NP] straight to DRAM,
    eliminating the on-device epilogue + DRAM transpose bounce entirely.
Data parallel over points: each core handles 8192 points; no collectives.
"""

import numpy as np

N_TOTAL = 65536
K_TOTAL = 512
N_CORES = 8
NP_CORE = N_TOTAL // N_CORES        # 8192 points per core
CHUNK = 512                         # points per matmul (PSUM bank = 512 fp32)
N_CHUNKS = NP_CORE // CHUNK         # 16
N_GROUPS = N_CHUNKS // 2            # chunk pairs sharing stationary loads
KBLK = K_TOTAL // 128               # 4 wavelet blocks of 128
NFEAT = 11                          # features per point
NST = 32                            # stacked contraction rows (ones-lo dropped)

_COMPILED = {}


def _split16(a):
    """Split fp32 into fp16 hi + fp16 lo (hi+lo carries ~21 mantissa bits)."""
    a = np.ascontiguousarray(a, np.float32)
    hi = a.astype(np.float16)
    lo = np.float32(a - hi.astype(np.float32)).astype(np.float16)
    return hi, lo


def _stack32(L):
    """[11,n] fp32 coeffs -> [32,n] fp16 stack [Lh; Ll; Lh[:10]]."""
    Lh, Ll = _split16(L)
    return np.concatenate([Lh, Ll, Lh[:NFEAT - 1]], axis=0)


def _build_program():
    import concourse.bacc as bacc
    import concourse.mybir as mybir
    import concourse.tile as tile

    f32 = mybir.dt.float32
    f16 = mybir.dt.float16
    AF = mybir.ActivationFunctionType

    nc = bacc.Bacc("TRN2", target_bir_lowering=False, debug=False)

    # fst: feature stack, host-replicated to all four 32-row groups
    fst_d = nc.dram_tensor("fst", [4 * NST, NP_CORE], f16, kind="ExternalInput")
    # lst: rows 0-31 Ls-stack, 32-63 Lt-stack, 64-95 Ls, 96-127 Lt;
    # columns grouped by k-block
    lst_d = nc.dram_tensor("lst", [128, K_TOTAL], f16, kind="ExternalInput")
    loh_d = nc.dram_tensor("loh", [128, KBLK * 10], f16, kind="ExternalInput")
    out_d = nc.dram_tensor("out", [10, NP_CORE], f32, kind="ExternalOutput")

    with tile.TileContext(nc) as tc:
        with (
            tc.tile_pool(name="persist", bufs=1) as pp,
            tc.tile_pool(name="epool", bufs=4) as ep,
            tc.tile_pool(name="psum_s", bufs=4, space="PSUM") as psps,
            tc.tile_pool(name="psum_t", bufs=3, space="PSUM") as pspt,
            tc.tile_pool(name="psum_out", bufs=1, space="PSUM") as pso,
            tc.tile_pool(name="wpool", bufs=24) as wpool,
        ):
            lst_t = pp.tile([128, K_TOTAL], f16, tag="lst")
            nc.gpsimd.dma_start(lst_t[:], lst_d[:])
            loh_t = pp.tile([128, KBLK * 10], f16, tag="loh")
            nc.scalar.dma_start(loh_t[:], loh_d[:])

            # persistent feature stack (host-replicated across row groups);
            # point-slices fan out over three DMA queues so early chunks are
            # never starved
            f_all = pp.tile([4 * NST, NP_CORE], f16, tag="f_all")
            bounds = [0, CHUNK, 2 * CHUNK, 4 * CHUNK] + [
                q * NP_CORE // 8 for q in range(3, 9)]
            engs = [nc.sync, nc.gpsimd, nc.scalar, nc.sync, nc.gpsimd,
                    nc.sync, nc.gpsimd, nc.sync, nc.gpsimd]
            for q in range(len(bounds) - 1):
                qs = slice(bounds[q], bounds[q + 1])
                engs[q].dma_start(f_all[:, qs], fst_d[:, qs])

            # warm the EXP activation-table load during the initial DMAs so
            # it is off the critical path of the first real exp
            warm = pp.tile([128, 1], f32, tag="warm")
            nc.gpsimd.memset(warm[:], 0.0)
            nc.scalar.activation(warm[:], warm[:], AF.Exp, scale=-0.5)

            # reduced rows R, staged in SBUF then DMA'd out per chunk-pair
            r_rows = pp.tile([10, NP_CORE], f32, tag="r_rows")

            pending = []   # (group, w tiles, po tile or None)

            def emit_outs(kbs):
                # emit one half of the previous group's output matmuls;
                # chunk cA accumulates at PE col group 0 (psum partitions
                # 0-9), cB at col group 32 (partitions 32-41): the paired
                # output matmuls run concurrently in the array and both
                # fit one PSUM bank
                if not pending:
                    return
                t0, w_ts, po = pending[-1]
                if po is None:
                    po = pso.tile([42, CHUNK], f32, tag="po", name=f"po{t0}")
                    pending[-1] = (t0, w_ts, po)
                po_j = (po[0:10, :], po[32:42, :])
                for kb in kbs:
                    for j in range(2):
                        nc.tensor.matmul(
                            po_j[j],
                            loh_t[:, kb * 10:(kb + 1) * 10],
                            w_ts[kb][j],
                            start=(kb == 0), stop=(kb == KBLK - 1),
                            tile_position=(0, 32 * j))
                if kbs[-1] != KBLK - 1:
                    return
                pending.pop()
                # drain: one chunk each on scalar and vector (gpsimd
                # cannot read PSUM)
                nc.scalar.copy(
                    r_rows[:, 2 * t0 * CHUNK:(2 * t0 + 1) * CHUNK], po_j[0])
                nc.vector.tensor_copy(
                    r_rows[:, (2 * t0 + 1) * CHUNK:(2 * t0 + 2) * CHUNK],
                    po_j[1])
                nc.sync.dma_start(
                    out_d[:, 2 * t0 * CHUNK:(2 * t0 + 2) * CHUNK],
                    r_rows[:, 2 * t0 * CHUNK:(2 * t0 + 2) * CHUNK])

            for t in range(N_GROUPS):
                cA, cB = 2 * t, 2 * t + 1
                fA = f_all[:, cA * CHUNK:(cA + 1) * CHUNK]
                fB = f_all[:, cB * CHUNK:(cB + 1) * CHUNK]
                w_ts = [[None, None] for _ in range(KBLK)]
                for p in range(KBLK // 2):
                    kbs = (2 * p, 2 * p + 1)
                    # interleave the two k-blocks of a pair so consecutive
                    # matmuls rotate through all four PE row groups
                    # (q0,q1,q2,q3) -> up to 4 matmuls in flight at once
                    xs = {}
                    ys = {}
                    for kb in kbs:
                        for j in range(2):
                            xs[kb, j] = psps.tile(
                                [128, CHUNK], f32, tag="ps_s",
                                name=f"s{t}_{kb}_{j}")
                            ys[kb, j] = pspt.tile(
                                [128, CHUNK], f32, tag="ps_t",
                                name=f"t{t}_{kb}_{j}")
                    for j, f_t in ((0, fA), (1, fB)):
                        for kb in kbs:
                            gs, gt = 2 * (kb % 2), 2 * (kb % 2) + 1
                            ks = slice(kb * 128, (kb + 1) * 128)
                            nc.tensor.matmul(
                                xs[kb, j][:],
                                lst_t[32 * gs:32 * (gs + 1), ks],
                                f_t[32 * gs:32 * (gs + 1), :],
                                start=True, stop=True,
                                tile_position=(32 * gs, 0))
                            nc.tensor.matmul(
                                ys[kb, j][:],
                                lst_t[32 * gt:32 * (gt + 1), ks],
                                f_t[32 * gt:32 * (gt + 1), :],
                                start=True, stop=True,
                                tile_position=(32 * gt, 0))
                    for j in range(2):
                        for kb in kbs:
                            e_t = ep.tile([128, CHUNK], f32, tag="e",
                                          name=f"e{t}_{kb}_{j}")
                            nc.scalar.activation(
                                e_t[:], xs[kb, j][:], AF.Exp, scale=-0.5)
                            w_t = wpool.tile([128, CHUNK], f16, tag="w",
                                             name=f"w{t}_{kb}_{j}")
                            nc.vector.tensor_mul(
                                w_t[:], ys[kb, j][:], e_t[:])
                            w_ts[kb][j] = w_t[:]
                    # previous group's output matmuls fill the PE while this
                    # block's exp/W-mul chain catches up
                    emit_outs([2 * p, 2 * p + 1])
                pending.append((t, w_ts, None))
            emit_outs([0, 1])
            emit_outs([2, 3])
    nc.compile()
    return nc


def _get_program():
    if "nc" not in _COMPILED:
        _COMPILED["nc"] = _build_program()
    return _COMPILED["nc"]


def _host_prep(x, y, z, wx, bx, wy, by, wz, bz, coeff):
    """Build per-core input maps (features + coefficient matrices)."""
    f8 = np.float64
    wx64, bx64 = wx.astype(f8), bx.astype(f8)
    wy64, by64 = wy.astype(f8), by.astype(f8)
    wz64, bz64 = wz.astype(f8), bz.astype(f8)
    c64 = coeff.astype(f8)
    sc = np.sqrt(np.clip(wx64 * wy64 * wz64, 1e-12, None))
    Z = np.zeros_like(wx64)

    # s = xt^2 + yt^2 + zt^2 over features [x2,y2,z2,xyz,xy,xz,yz,x,y,z,1]
    Ls = np.stack([
        wx64 ** 2, wy64 ** 2, wz64 ** 2, Z, Z, Z, Z,
        -2 * wx64 * bx64, -2 * wy64 * by64, -2 * wz64 * bz64,
        bx64 ** 2 + by64 ** 2 + bz64 ** 2,
    ]).astype(np.float32)                      # [11, K]
    # T3 = xt*yt*zt
    Lt = np.stack([
        Z, Z, Z,
        wx64 * wy64 * wz64, -wx64 * wy64 * bz64, -wx64 * by64 * wz64,
        -bx64 * wy64 * wz64, wx64 * by64 * bz64, bx64 * wy64 * bz64,
        bx64 * by64 * wz64, -bx64 * by64 * bz64,
    ]).astype(np.float32)                      # [11, K]
    b1 = c64 * sc * wx64 ** 2
    b2 = c64 * sc * wy64 ** 2
    b3 = c64 * sc * wz64 ** 2
    Lo = np.stack([
        -c64 * sc,
        -b1 * wx64 ** 2, 2 * b1 * wx64 * bx64, b1 * (3 - bx64 ** 2),
        -b2 * wy64 ** 2, 2 * b2 * wy64 * by64, b2 * (3 - by64 ** 2),
        -b3 * wz64 ** 2, 2 * b3 * wz64 * bz64, b3 * (3 - bz64 ** 2),
    ], axis=1).astype(np.float32)              # [K, 10]

    Ls32 = _stack32(Ls)                        # [32, K] fp16
    Lt32 = _stack32(Lt)
    lst_pack = np.concatenate([Ls32, Lt32, Ls32, Lt32], axis=0)  # [128, K]
    Loh = Lo.astype(np.float16)
    loh_pack = np.concatenate(
        [Loh[kb * 128:(kb + 1) * 128] for kb in range(KBLK)], axis=1)  # [128, 40]

    in_maps = []
    for cid in range(N_CORES):
        sl = slice(cid * NP_CORE, (cid + 1) * NP_CORE)
        xs, ys, zs = (np.ascontiguousarray(a[sl], np.float32) for a in (x, y, z))
        F = np.stack([
            xs * xs, ys * ys, zs * zs, xs * ys * zs, xs * ys, xs * zs,
            ys * zs, xs, ys, zs, np.ones_like(xs),
        ]).astype(np.float32)                  # [11, NP_CORE]
        Fh, Fl = _split16(F)
        fst1 = np.concatenate([Fh, Fh, Fl[:NFEAT - 1]], axis=0)  # [32, NP]
        fst = np.concatenate([fst1] * 4, axis=0)                 # [128, NP]
        in_maps.append({"fst": fst, "lst": lst_pack, "loh": loh_pack})
    return in_maps


def _run_device(in_maps, trace=False):
    from concourse.bass_utils import run_bass_kernel_spmd
    nc = _get_program()
    last_err = None
    for _attempt in range(3):
        try:
            return run_bass_kernel_spmd(
                nc, in_maps, list(range(N_CORES)), trace=trace)
        except Exception as ex:  # transient NRT device errors recover on retry
            last_err = ex
    raise last_err


def kernel(x, y, z, wx, bx, wy, by, wz, bz, coeff, bias, _trace=False):
    x, y, z = (np.asarray(a, np.float32) for a in (x, y, z))
    in_maps = _host_prep(
        x, y, z,
        *(np.asarray(a, np.float32) for a in (wx, bx, wy, by, wz, bz, coeff)))
    res = _run_device(in_maps, trace=_trace)
    R = np.concatenate(
        [res.results[cid]["out"] for cid in range(N_CORES)], axis=1)  # [10, N]
    bias_f = np.float32(np.asarray(bias))
    x64, y64, z64 = (a.astype(np.float64) for a in (x, y, z))
    R64 = R.astype(np.float64)
    output = (R64[0] + np.float64(bias_f)).astype(np.float32)
    d2x = (x64 * x64 * R64[1] + x64 * R64[2] + R64[3]).astype(np.float32)
    d2y = (y64 * y64 * R64[4] + y64 * R64[5] + R64[6]).astype(np.float32)
    d2z = (z64 * z64 * R64[7] + z64 * R64[8] + R64[9]).astype(np.float32)
    if _trace:
        kernel._last_results = res
    return (output, d2x, d2y, d2z)
